# revision 10
# baseline (speedup 1.0000x reference)
"""Trainium2 Bass kernel for nn_PitchLoss (segment_reduce).

Math: for each note k with frame range [a_k, b_k), the reference builds a
dense (T, N) mask and computes per-note means of gen_f0 / t_f0 over the
range, then loss = mean((|mean_gen - mean_ref| > 0.5)).

Since each note is a contiguous frame range, per-note sums are prefix-sum
differences: with d = gen_f0 - t_f0 and cse[x] = sum(d[0:x]),
    |mean_gen_k - mean_ref_k| = |cse[b_k] - cse[a_k]| / (b_k - a_k)
so  verdict_k = (b_k > a_k) & (|cse[b_k] - cse[a_k]| > 0.5 * (b_k - a_k))
which also reproduces the reference's empty-segment NaN > 0.5 == False.

Sharding: notes across 8 cores (128 notes/core); gen_f0/t_f0 replicated.
Per core: O(T) scan + one-hot matmul gather of cse at 256 indices.
Host sums the 1024 binary verdicts -> loss (exact, /1024 is a pow2).
"""

import numpy as np

import concourse.bacc as bacc
import concourse.bass as bass
import concourse.tile as tile
from concourse import mybir
from concourse.bass_utils import run_bass_kernel_spmd

T = 32768           # frames
N = 1024            # notes
NCORES = 8
NPC = N // NCORES   # notes per core
P = 128             # partitions
F = T // P          # 256 frames per partition row
FP1 = F + 1         # 257: cse columns (f in [0, 256])
FC = F + 2          # 258: + partition-index column
DT = mybir.dt.float32
I32 = mybir.dt.int32
ALU = mybir.AluOpType


def build_nc():
    nc = bacc.Bacc("TRN2", target_bir_lowering=False, debug=False)
    gen = nc.dram_tensor("gen_f0", [T], DT, kind="ExternalInput")
    ref = nc.dram_tensor("t_f0", [T], DT, kind="ExternalInput")
    on = nc.dram_tensor("onset", [NPC], I32, kind="ExternalInput")
    off = nc.dram_tensor("offset", [NPC], I32, kind="ExternalInput")
    out = nc.dram_tensor("verdict", [NPC], DT, kind="ExternalOutput")

    with tile.TileContext(nc) as tc, \
         tc.tile_pool(name="sb", bufs=1) as sb, \
         tc.tile_pool(name="ps", bufs=1, space="PSUM") as ps:

        # ---- constant index tiles (f32 exact: all values < 2^24) ----
        iota_f = sb.tile([P, FP1], DT, tag="iota_f")  # [p, f] = f
        nc.gpsimd.iota(iota_f[:], pattern=[[1, FP1]], base=0,
                       channel_multiplier=0, allow_small_or_imprecise_dtypes=True)
        p256 = sb.tile([P, 1], DT, tag="p256")        # 256*p
        nc.gpsimd.iota(p256[:], pattern=[[0, 1]], base=0,
                       channel_multiplier=F, allow_small_or_imprecise_dtypes=True)
        p256e = sb.tile([P, 1], DT, tag="p256e")      # 256*p + 256
        nc.gpsimd.iota(p256e[:], pattern=[[0, 1]], base=F,
                       channel_multiplier=F, allow_small_or_imprecise_dtypes=True)
        pidx = sb.tile([P, 1], DT, tag="pidx")        # p
        nc.gpsimd.iota(pidx[:], pattern=[[0, 1]], base=0,
                       channel_multiplier=1, allow_small_or_imprecise_dtypes=True)
        ones = sb.tile([P, P], DT, tag="ones")
        nc.vector.memset(ones[:], 1.0)
        stri = sb.tile([P, P], DT, tag="stri")        # [k, m] = 1 iff m > k
        nc.gpsimd.affine_select(stri[:], ones[:], pattern=[[1, P]], base=0,
                                channel_multiplier=-1,
                                compare_op=ALU.is_gt, fill=0.0)

        # ---- d = gen - ref, laid out [p, f] with t = 256p + f ----
        g = sb.tile([P, F], DT, tag="g")
        r = sb.tile([P, F], DT, tag="r")
        nc.gpsimd.dma_start(out=g[:], in_=gen[:].rearrange("(p f) -> p f", p=P))
        nc.gpsimd.dma_start(out=r[:], in_=ref[:].rearrange("(p f) -> p f", p=P))
        d = sb.tile([P, F], DT, tag="d")
        nc.vector.tensor_sub(d[:], g[:], r[:])

        # inclusive per-row scan, then exclusive cross-row offsets via
        # strictly-triangular matmul on the row totals
        sc = sb.tile([P, F], DT, tag="sc")
        nc.vector.tensor_tensor_scan(sc[:], d[:], d[:], 0.0,
                                     op0=ALU.add, op1=ALU.bypass)
        roff = ps.tile([P, 1], DT, tag="roff")
        nc.tensor.matmul(roff[:], stri[:], sc[:, F - 1:F], start=True, stop=True)

        # cse[p, f] = exclusive cumsum of d at t = 256p + f, f in [0, 256];
        # col 257 carries the partition index for the gather's f-recovery
        cse = sb.tile([P, FC], DT, tag="cse")
        nc.scalar.copy(cse[:, 0:1], roff[:])
        nc.vector.tensor_scalar(cse[:, 1:FP1], sc[:], roff[:], None, op0=ALU.add)
        nc.scalar.copy(cse[:, FP1:FC], pidx[:])

        # ---- gather cse[x] for a 128-vector of indices x in [0, 32768] ----
        def gather(vecdram, tag):
            xi = sb.tile([P, 1], I32, tag=f"xi_{tag}")
            nc.gpsimd.dma_start(out=xi[:], in_=vecdram[:].rearrange("(p f) -> p f", f=1))
            xf = sb.tile([P, 1], DT, tag=f"xf_{tag}")
            nc.scalar.copy(xf[:], xi[:])  # int32 -> f32 (exact)

            # x broadcast to all partitions (DMA stride-0), then one-hot over
            # partitions: onep[p, k] = (x_k >= 256p) & (x_k < 256p + 256)
            xbi = sb.tile([P, NPC], I32, tag=f"xbi_{tag}")
            src = vecdram[:]
            bcast = bass.AP(tensor=src.tensor, offset=src.offset,
                            ap=[[0, P]] + [list(pair) for pair in src.ap])
            nc.gpsimd.dma_start(out=xbi[:], in_=bcast)
            xb = sb.tile([P, NPC], DT, tag=f"xb_{tag}")
            nc.scalar.copy(xb[:], xbi[:])
            lt = sb.tile([P, NPC], DT, tag=f"lt_{tag}")
            nc.vector.tensor_scalar(lt[:], xb[:], p256e[:], None, op0=ALU.is_lt)
            onep = sb.tile([P, NPC], DT, tag=f"onep_{tag}")
            nc.vector.scalar_tensor_tensor(onep[:], xb[:], p256[:], lt[:],
                                           op0=ALU.is_ge, op1=ALU.mult)
            # x == 32768 (b_k == T) has no one-hot row: handled below by
            # clamping to row 127 / f = 256, whose cse entry is the total sum.
            # onep for x=32768 is all-zero -> rg row = 0, p_gathered = 0.
            # Fix: add the "past the end" row via max with row-127 predicate:
            ovf = sb.tile([P, NPC], DT, tag=f"ovf_{tag}")
            # ovf[p, k] = (x_k >= 256p + 256) & (p == 127) -> only x == 32768
            # Simpler: ovf[p,k] = (x_k is_ge 32768) * (p256 == 256*127)
            nc.vector.tensor_scalar(ovf[:], xb[:], float(T), None, op0=ALU.is_ge)
            islast = sb.tile([P, 1], DT, tag=f"il_{tag}")
            nc.vector.tensor_scalar(islast[:], p256[:], float(T - F), None,
                                    op0=ALU.is_equal)
            nc.vector.tensor_scalar(ovf[:], ovf[:], islast[:], None, op0=ALU.mult)
            nc.vector.tensor_add(onep[:], onep[:], ovf[:])

            rg = ps.tile([P, FC], DT, tag=f"rg_{tag}")
            nc.tensor.matmul(rg[:], onep[:], cse[:], start=True, stop=True)

            # f = x - 256 * p_gathered  (for x = 32768: p = 127 -> f = 256)
            fcol = sb.tile([P, 1], DT, tag=f"fcol_{tag}")
            nc.vector.scalar_tensor_tensor(fcol[:], rg[:, FP1:FC], -float(F),
                                           xf[:], op0=ALU.mult, op1=ALU.add)
            onef = sb.tile([P, FP1], DT, tag=f"onef_{tag}")
            nc.vector.tensor_scalar(onef[:], iota_f[:], fcol[:], None,
                                    op0=ALU.is_equal)
            scr = sb.tile([P, FP1], DT, tag=f"scr_{tag}")
            val = sb.tile([P, 1], DT, tag=f"val_{tag}")
            nc.vector.scalar_tensor_tensor(scr[:], rg[:, 0:FP1], 1.0, onef[:],
                                           op0=ALU.mult, op1=ALU.mult,
                                           accum_out=val[:])
            return xf, val

        af, va = gather(on, "a")
        bf, vb = gather(off, "b")

        # ---- verdict = (b > a) & (|cse[b] - cse[a]| > 0.5 * (b - a)) ----
        delta = sb.tile([P, 1], DT, tag="delta")
        nc.vector.tensor_sub(delta[:], vb[:], va[:])
        absd = sb.tile([P, 1], DT, tag="absd")
        nc.vector.scalar_tensor_tensor(absd[:], delta[:], -1.0, delta[:],
                                       op0=ALU.mult, op1=ALU.max)
        msum = sb.tile([P, 1], DT, tag="msum")
        nc.vector.tensor_sub(msum[:], bf[:], af[:])
        cmp = sb.tile([P, 1], DT, tag="cmp")
        nc.vector.scalar_tensor_tensor(cmp[:], msum[:], 0.5, absd[:],
                                       op0=ALU.mult, op1=ALU.is_lt)
        pos = sb.tile([P, 1], DT, tag="pos")
        nc.vector.tensor_scalar(pos[:], msum[:], 0.0, None, op0=ALU.is_gt)
        v = sb.tile([P, 1], DT, tag="v")
        nc.vector.tensor_mul(v[:], cmp[:], pos[:])
        nc.sync.dma_start(out=out[:].rearrange("(p f) -> p f", f=1), in_=v[:])
    nc.finalize()
    return nc


_NC_CACHE = None


def _get_nc():
    global _NC_CACHE
    if _NC_CACHE is None:
        _NC_CACHE = build_nc()
    return _NC_CACHE


def _run(inputs, **kwargs):
    gen = np.ascontiguousarray(inputs["gen_f0"], dtype=np.float32)
    ref = np.ascontiguousarray(inputs["t_f0"], dtype=np.float32)
    on = np.ascontiguousarray(inputs["onset_times"], dtype=np.int32)
    off = np.ascontiguousarray(inputs["offset_times"], dtype=np.int32)

    nc = _get_nc()
    in_maps = [
        {
            "gen_f0": gen,
            "t_f0": ref,
            "onset": np.ascontiguousarray(on[c * NPC:(c + 1) * NPC]),
            "offset": np.ascontiguousarray(off[c * NPC:(c + 1) * NPC]),
        }
        for c in range(NCORES)
    ]
    return run_bass_kernel_spmd(nc, in_maps, core_ids=list(range(NCORES)),
                                **kwargs)


def kernel(**inputs):
    res = _run(inputs)
    verdicts = np.concatenate([res.results[c]["verdict"] for c in range(NCORES)])
    return np.asarray(verdicts.sum() / np.float32(N), dtype=np.float32)


# revision 11
# speedup vs baseline: 1.0160x; 1.0160x over previous
"""Trainium2 Bass kernel for nn_PitchLoss (segment_reduce).

Math: for each note k with frame range [a_k, b_k), the reference builds a
dense (T, N) mask and computes per-note means of gen_f0 / t_f0 over the
range, then loss = mean((|mean_gen - mean_ref| > 0.5)).

Since each note is a contiguous frame range, per-note sums are prefix-sum
differences: with d = gen_f0 - t_f0 and cse[x] = sum(d[0:x]),
    |mean_gen_k - mean_ref_k| = |cse[b_k] - cse[a_k]| / (b_k - a_k)
so  verdict_k = (b_k > a_k) & (|cse[b_k] - cse[a_k]| > 0.5 * (b_k - a_k))
which also reproduces the reference's empty-segment NaN > 0.5 == False.

Sharding: notes across 8 cores (128 notes/core); gen_f0/t_f0 replicated.
Per core: O(T) scan + one-hot matmul gather of cse at 256 indices.
Host packs inputs into two DRAM tensors (f0cat, onoff) to minimize DMAs,
and sums the 1024 binary verdicts -> loss (exact, /1024 is a pow2).
"""

import numpy as np

import concourse.bacc as bacc
import concourse.bass as bass
import concourse.tile as tile
from concourse import mybir
from concourse.bass_utils import run_bass_kernel_spmd

T = 32768           # frames
N = 1024            # notes
NCORES = 8
NPC = N // NCORES   # notes per core
P = 128             # partitions
F = T // P          # 256 frames per partition row
FP1 = F + 1         # 257: cse columns (f in [0, 256])
FC = F + 2          # 258: + row-base (256p) column
K2 = 2 * NPC        # 256: onsets ++ offsets
DT = mybir.dt.float32
I32 = mybir.dt.int32
ALU = mybir.AluOpType


def build_nc():
    nc = bacc.Bacc("TRN2", target_bir_lowering=False, debug=False)
    f0cat = nc.dram_tensor("f0cat", [2 * T], DT, kind="ExternalInput")
    onoff = nc.dram_tensor("onoff", [K2], I32, kind="ExternalInput")
    out = nc.dram_tensor("verdict", [NPC], DT, kind="ExternalOutput")

    with tile.TileContext(nc) as tc, \
         tc.tile_pool(name="sb", bufs=1) as sb, \
         tc.tile_pool(name="ps", bufs=1, space="PSUM") as ps:

        # ---- constant index tiles (f32 exact: all values < 2^24) ----
        iota_f = sb.tile([P, FP1], DT, tag="iota_f")  # [p, f] = f
        nc.gpsimd.iota(iota_f[:], pattern=[[1, FP1]], base=0,
                       channel_multiplier=0, allow_small_or_imprecise_dtypes=True)
        p256 = sb.tile([P, 1], DT, tag="p256")        # 256*p
        nc.gpsimd.iota(p256[:], pattern=[[0, 1]], base=0,
                       channel_multiplier=F, allow_small_or_imprecise_dtypes=True)
        p256e = sb.tile([P, 1], DT, tag="p256e")      # 256*p + 256
        nc.gpsimd.iota(p256e[:], pattern=[[0, 1]], base=F,
                       channel_multiplier=F, allow_small_or_imprecise_dtypes=True)
        ones = sb.tile([P, P], DT, tag="ones")
        nc.vector.memset(ones[:], 1.0)
        stri = sb.tile([P, P], DT, tag="stri")        # [k, m] = 1 iff m > k
        nc.gpsimd.affine_select(stri[:], ones[:], pattern=[[1, P]], base=0,
                                channel_multiplier=-1,
                                compare_op=ALU.is_gt, fill=0.0)

        # ---- one DMA for both f0 vectors: fr[p, s, f] = f0_s[256p + f] ----
        fr = sb.tile([P, 2, F], DT, tag="fr")
        f0_ap = bass.AP(tensor=f0cat[:].tensor, offset=0,
                        ap=[[F, P], [T, 2], [1, F]])
        nc.sync.dma_start(out=fr[:], in_=f0_ap)

        # fused diff + inclusive scan: state = (gen + state) - ref
        sc = sb.tile([P, F], DT, tag="sc")
        nc.vector.tensor_tensor_scan(sc[:], fr[:, 0, :], fr[:, 1, :], 0.0,
                                     op0=ALU.add, op1=ALU.subtract)
        # exclusive cross-row offsets via strictly-triangular matmul
        roff = ps.tile([P, 1], DT, tag="roff")
        nc.tensor.matmul(roff[:], stri[:], sc[:, F - 1:F], start=True, stop=True)

        # cse[p, f] = exclusive cumsum at t = 256p + f, f in [0, 256];
        # col 257 = 256p (row base, used to recover f after the row gather)
        cse = sb.tile([P, FC], DT, tag="cse")
        nc.vector.tensor_copy(cse[:, 0:1], roff[:])
        nc.vector.tensor_scalar(cse[:, 1:FP1], sc[:], roff[:], None, op0=ALU.add)
        nc.vector.tensor_copy(cse[:, FP1:FC], p256[:])

        # ---- index loads: columns (onset | offset) and all-partition bcast --
        oc = sb.tile([P, 2], I32, tag="oc")
        oc_ap = bass.AP(tensor=onoff[:].tensor, offset=0,
                        ap=[[1, P], [NPC, 2]])
        nc.sync.dma_start(out=oc[:], in_=oc_ap)
        xf = sb.tile([P, 2], DT, tag="xf")
        nc.vector.tensor_copy(xf[:], oc[:])           # int32 -> f32 (exact)

        obi = sb.tile([P, K2], I32, tag="obi")
        ob_ap = bass.AP(tensor=onoff[:].tensor, offset=0,
                        ap=[[0, P], [1, K2]])
        nc.sync.dma_start(out=obi[:], in_=ob_ap)
        xb = sb.tile([P, K2], DT, tag="xb")
        nc.gpsimd.tensor_copy(xb[:], obi[:])          # int32 -> f32 (exact)

        # ---- one-hot over partitions for both index sets at once ----
        # onep[p, k] = (x_k >= 256p) & (x_k < 256p + 256), plus the x == T
        # overflow pinned to row 127 (its cse col-256 entry is the total)
        lt = sb.tile([P, K2], DT, tag="lt")
        nc.vector.tensor_scalar(lt[:], xb[:], p256e[:], None, op0=ALU.is_lt)
        onep = sb.tile([P, K2], DT, tag="onep")
        nc.vector.scalar_tensor_tensor(onep[:], xb[:], p256[:], lt[:],
                                       op0=ALU.is_ge, op1=ALU.mult)
        islast = sb.tile([P, 1], DT, tag="islast")
        nc.vector.tensor_scalar(islast[:], p256[:], float(T - F), None,
                                op0=ALU.is_equal)
        ovf = sb.tile([P, K2], DT, tag="ovf")
        nc.vector.tensor_scalar(ovf[:], xb[:], float(T), None, op0=ALU.is_ge)
        onep2 = sb.tile([P, K2], DT, tag="onep2")
        nc.vector.scalar_tensor_tensor(onep2[:], ovf[:], islast[:], onep[:],
                                       op0=ALU.mult, op1=ALU.add)

        # ---- row-gather matmuls + in-row select ----
        def gather(ksl, xfcol, tag):
            rg = ps.tile([P, FC], DT, tag=f"rg_{tag}")
            nc.tensor.matmul(rg[:], onep2[:, ksl], cse[:], start=True, stop=True)
            fcol = sb.tile([P, 1], DT, tag=f"fcol_{tag}")
            nc.vector.scalar_tensor_tensor(fcol[:], rg[:, FP1:FC], -1.0,
                                           xfcol, op0=ALU.mult, op1=ALU.add)
            onef = sb.tile([P, FP1], DT, tag=f"onef_{tag}")
            nc.vector.tensor_scalar(onef[:], iota_f[:], fcol[:], None,
                                    op0=ALU.is_equal)
            scr = sb.tile([P, FP1], DT, tag=f"scr_{tag}")
            val = sb.tile([P, 1], DT, tag=f"val_{tag}")
            nc.vector.scalar_tensor_tensor(scr[:], rg[:, 0:FP1], 1.0, onef[:],
                                           op0=ALU.mult, op1=ALU.mult,
                                           accum_out=val[:])
            return val

        va = gather(slice(0, NPC), xf[:, 0:1], "a")
        vb = gather(slice(NPC, K2), xf[:, 1:2], "b")

        # ---- verdict = (b > a) & (|cse[b] - cse[a]| > 0.5 * (b - a)) ----
        delta = sb.tile([P, 1], DT, tag="delta")
        nc.vector.tensor_sub(delta[:], vb[:], va[:])
        absd = sb.tile([P, 1], DT, tag="absd")
        nc.vector.scalar_tensor_tensor(absd[:], delta[:], -1.0, delta[:],
                                       op0=ALU.mult, op1=ALU.max)
        msum = sb.tile([P, 1], DT, tag="msum")
        nc.vector.tensor_sub(msum[:], xf[:, 1:2], xf[:, 0:1])
        cmp = sb.tile([P, 1], DT, tag="cmp")
        nc.vector.scalar_tensor_tensor(cmp[:], msum[:], 0.5, absd[:],
                                       op0=ALU.mult, op1=ALU.is_lt)
        pos = sb.tile([P, 1], DT, tag="pos")
        nc.vector.tensor_scalar(pos[:], msum[:], 0.0, None, op0=ALU.is_gt)
        v = sb.tile([P, 1], DT, tag="v")
        nc.vector.tensor_mul(v[:], cmp[:], pos[:])
        nc.sync.dma_start(out=out[:].rearrange("(p f) -> p f", f=1), in_=v[:])
    nc.finalize()
    return nc


_NC_CACHE = None


def _get_nc():
    global _NC_CACHE
    if _NC_CACHE is None:
        _NC_CACHE = build_nc()
    return _NC_CACHE


def _run(inputs, **kwargs):
    gen = np.ascontiguousarray(inputs["gen_f0"], dtype=np.float32)
    ref = np.ascontiguousarray(inputs["t_f0"], dtype=np.float32)
    on = np.ascontiguousarray(inputs["onset_times"], dtype=np.int32)
    off = np.ascontiguousarray(inputs["offset_times"], dtype=np.int32)

    f0cat = np.concatenate([gen, ref])
    nc = _get_nc()
    in_maps = [
        {
            "f0cat": f0cat,
            "onoff": np.concatenate([on[c * NPC:(c + 1) * NPC],
                                     off[c * NPC:(c + 1) * NPC]]),
        }
        for c in range(NCORES)
    ]
    return run_bass_kernel_spmd(nc, in_maps, core_ids=list(range(NCORES)),
                                **kwargs)


def kernel(**inputs):
    res = _run(inputs)
    verdicts = np.concatenate([res.results[c]["verdict"] for c in range(NCORES)])
    return np.asarray(verdicts.sum() / np.float32(N), dtype=np.float32)
